# revision 1
# baseline (speedup 1.0000x reference)
"""BERT embedding (token/type/position gather + LayerNorm) on 8 Trainium2 cores.

Sharding: data-parallel over batch — core c handles sequences [4c, 4c+4),
i.e. 2048 tokens. Each core holds the full token embedding table and
gathers its rows with indirect DMA; type/position embeddings are folded in
(host folds type_W[0] into the position table; the device adds
t * (type_W[1] - type_W[0]) via a per-partition-scaled activation).
LayerNorm runs per token tile with bn_stats/bn_aggr.
"""
import numpy as np

import concourse.bacc as bacc
import concourse.bass as bass
import concourse.tile as tile
from concourse import mybir
from concourse.bass_utils import run_bass_kernel_spmd

P = 128
N_CORES = 8
B, S, V, H, T = 32, 512, 30522, 1024, 2
EPS = 1e-5
B_PER_CORE = B // N_CORES       # 4 sequences per core
N_TOK = B_PER_CORE * S          # 2048 tokens per core
NT = N_TOK // P                 # 16 token tiles per core
Q = S // P                      # 4 position quarters

F32 = mybir.dt.float32
I32 = mybir.dt.int32

_cache: dict = {}


def _build(apply_ln: bool):
    nc = bacc.Bacc(None, target_bir_lowering=False)
    tok_w = nc.declare_dram_parameter("tok_w", [V, H], F32, isOutput=False)
    ids = nc.declare_dram_parameter("ids", [P, NT], I32, isOutput=False)
    tt = nc.declare_dram_parameter("tt", [P, NT], F32, isOutput=False)
    pos_c = nc.declare_dram_parameter("pos_c", [P, Q, H], F32, isOutput=False)
    delta = nc.declare_dram_parameter("delta", [1, H], F32, isOutput=False)
    if apply_ln:
        lnw = nc.declare_dram_parameter("lnw", [1, H], F32, isOutput=False)
        lnb = nc.declare_dram_parameter("lnb", [1, H], F32, isOutput=False)
    out_d = nc.declare_dram_parameter("out", [N_TOK, H], F32, isOutput=True)

    with tile.TileContext(nc) as tc:
        with (
            tc.tile_pool(name="singles", bufs=1) as singles,
            tc.tile_pool(name="work", bufs=4) as work,
            tc.tile_pool(name="stats", bufs=6) as stats_p,
        ):
            ids_sb = singles.tile([P, NT], I32)
            nc.sync.dma_start(out=ids_sb[:], in_=ids[:])
            tt_sb = singles.tile([P, NT], F32)
            nc.sync.dma_start(out=tt_sb[:], in_=tt[:])
            pos_sb = singles.tile([P, Q, H], F32)
            nc.sync.dma_start(out=pos_sb[:], in_=pos_c[:])
            delta_sb = singles.tile([P, H], F32)
            nc.gpsimd.dma_start(out=delta_sb[:], in_=delta[:].to_broadcast([P, H]))
            eps_sb = singles.tile([P, 1], F32)
            nc.vector.memset(eps_sb[:], EPS)
            if apply_ln:
                lnw_sb = singles.tile([P, H], F32)
                nc.gpsimd.dma_start(out=lnw_sb[:], in_=lnw[:].to_broadcast([P, H]))
                lnb_sb = singles.tile([P, H], F32)
                nc.gpsimd.dma_start(out=lnb_sb[:], in_=lnb[:].to_broadcast([P, H]))

            for j in range(NT):
                te = work.tile([P, H], F32, tag="te")
                nc.gpsimd.indirect_dma_start(
                    out=te[:],
                    out_offset=None,
                    in_=tok_w[:],
                    in_offset=bass.IndirectOffsetOnAxis(
                        ap=ids_sb[:, j:j + 1], axis=0
                    ),
                )
                # td = t * (type_W[1] - type_W[0]) on the scalar engine
                td = work.tile([P, H], F32, tag="td")
                nc.scalar.activation(
                    out=td[:],
                    in_=delta_sb[:],
                    func=mybir.ActivationFunctionType.Copy,
                    scale=tt_sb[:, j:j + 1],
                )
                nc.vector.tensor_add(out=te[:], in0=te[:], in1=pos_sb[:, j % Q, :])
                nc.vector.tensor_add(out=te[:], in0=te[:], in1=td[:])

                stats = stats_p.tile([P, 2, 6], F32, tag="st")
                nc.vector.bn_stats(out=stats[:, 0, :], in_=te[:, 0:512])
                nc.vector.bn_stats(out=stats[:, 1, :], in_=te[:, 512:H])
                mv = stats_p.tile([P, 2], F32, tag="mv")
                nc.vector.bn_aggr(out=mv[:], in_=stats[:])

                std = stats_p.tile([P, 1], F32, tag="sd")
                nc.scalar.activation(
                    out=std[:],
                    in_=mv[:, 1:2],
                    func=mybir.ActivationFunctionType.Sqrt,
                    bias=eps_sb[:],
                    scale=1.0,
                )
                nc.vector.reciprocal(out=std[:], in_=std[:])

                o = work.tile([P, H], F32, tag="o")
                nc.vector.tensor_scalar(
                    out=o[:],
                    in0=te[:],
                    scalar1=mv[:, 0:1],
                    scalar2=std[:],
                    op0=mybir.AluOpType.subtract,
                    op1=mybir.AluOpType.mult,
                )
                if apply_ln:
                    nc.vector.tensor_mul(out=o[:], in0=o[:], in1=lnw_sb[:])
                    nc.vector.tensor_add(out=o[:], in0=o[:], in1=lnb_sb[:])
                nc.sync.dma_start(out=out_d[j * P:(j + 1) * P, :], in_=o[:])
    nc.finalize()
    return nc


def _prepare_inputs(input_ids, token_type_ids, token_W, pos_W, type_W,
                    ln_w, ln_b, apply_ln):
    ids_np = np.asarray(input_ids).reshape(B, S)
    tt_np = np.asarray(token_type_ids).reshape(B, S)
    tok = np.ascontiguousarray(np.asarray(token_W, dtype=np.float32))
    pos = np.asarray(pos_W, dtype=np.float32)
    typ = np.asarray(type_W, dtype=np.float32)

    pos_comb = pos[:S] + typ[0]                                     # [S, H]
    pos_hw = np.ascontiguousarray(pos_comb.reshape(Q, P, H).transpose(1, 0, 2))
    delta = np.ascontiguousarray((typ[1] - typ[0]).reshape(1, H))

    in_maps = []
    for c in range(N_CORES):
        idc = ids_np[c * B_PER_CORE:(c + 1) * B_PER_CORE].reshape(N_TOK)
        ttc = tt_np[c * B_PER_CORE:(c + 1) * B_PER_CORE].reshape(N_TOK)
        m = {
            "tok_w": tok,
            "ids": np.ascontiguousarray(idc.reshape(NT, P).T.astype(np.int32)),
            "tt": np.ascontiguousarray(ttc.reshape(NT, P).T.astype(np.float32)),
            "pos_c": pos_hw,
            "delta": delta,
        }
        if apply_ln:
            m["lnw"] = np.ascontiguousarray(
                np.asarray(ln_w, dtype=np.float32).reshape(1, H))
            m["lnb"] = np.ascontiguousarray(
                np.asarray(ln_b, dtype=np.float32).reshape(1, H))
        in_maps.append(m)
    return in_maps


def _run(input_ids, token_type_ids, token_W, pos_W, type_W, ln_w, ln_b,
         trace=False):
    lnw = np.asarray(ln_w, dtype=np.float32).reshape(-1)
    lnb = np.asarray(ln_b, dtype=np.float32).reshape(-1)
    apply_ln = not (np.all(lnw == 1.0) and np.all(lnb == 0.0))

    nc = _cache.get(apply_ln)
    if nc is None:
        nc = _cache.setdefault(apply_ln, _build(apply_ln))
    in_maps = _prepare_inputs(input_ids, token_type_ids, token_W, pos_W,
                              type_W, ln_w, ln_b, apply_ln)
    res = run_bass_kernel_spmd(nc, in_maps, list(range(N_CORES)), trace=trace)
    out = np.concatenate(
        [res.results[c]["out"].reshape(B_PER_CORE, S, H) for c in range(N_CORES)],
        axis=0,
    )
    return out, res


def kernel(input_ids, token_type_ids, token_W, pos_W, type_W, ln_w, ln_b):
    out, _ = _run(input_ids, token_type_ids, token_W, pos_W, type_W,
                  ln_w, ln_b, trace=False)
    return out
